# revision 7
# baseline (speedup 1.0000x reference)
"""Multi-head attention (B=4, S=2048, E=1024, H=16, D=64) on 8 TRN2 NeuronCores.

Sharding: core c handles batch b = c//2 and heads [8*(c%2), 8*(c%2)+8) —
data parallel over batch, tensor parallel over heads. No collectives:
each core computes its own output slice, gathered on host.

Per-core algorithm (all matmuls in float32r = full-rate fp32):
  qT = (Wq_slice)^T-free matmul:  qT[f, s]  = sum_e Wq[e, f] * XqT[e, s]
  kT likewise; v[s, f] = sum_e XvT[e, s] * Wv[e, f]  (natural layout)
  per head h, sq-chunk j (512 wide):
    S^T[sk_blk, sq] = matmul(lhsT=kT_h[:, blk], rhs=qT_h[:, j])   (K=64)
    P = exp(S^T / 8)            (ACT, batched over 2 psum banks)
    ctx^T[0:64, sq] += matmul(lhsT=[v_h | 1], rhs=P)  -> row 64 = sum(P)
  output per core: [8 heads, 65, 2048]; host divides rows 0..63 by row 64,
  transposes, and scatters into the full [4, 2048, 1024] result.
"""

import numpy as np
from contextlib import ExitStack

import concourse.bass as bass
import concourse.tile as tile
from concourse import bacc
from concourse import mybir
from concourse.bass_utils import run_bass_kernel_spmd

F32 = mybir.dt.float32
F32R = mybir.dt.float32r
EXP = mybir.ActivationFunctionType.Exp

B, S, E = 4, 2048, 1024
H, D = 16, 64
HPC = 8            # heads per core
FPC = HPC * D      # 512 output features per core
N_CORES = 8
KC = E // 128      # contraction chunks
NJ = S // 512      # sq chunks
NT = S // 128      # sk blocks
SCALE = 0.125      # 1/sqrt(64)


def build_bass():
    nc = bacc.Bacc()
    xq = nc.declare_dram_parameter("xq_t", [E, S], F32R, isOutput=False)
    xk = nc.declare_dram_parameter("xk_t", [E, S], F32R, isOutput=False)
    xv = nc.declare_dram_parameter("xv_t", [E, S], F32R, isOutput=False)
    wq = nc.declare_dram_parameter("wq", [E, FPC], F32R, isOutput=False)
    wk = nc.declare_dram_parameter("wk", [E, FPC], F32R, isOutput=False)
    wv = nc.declare_dram_parameter("wv", [E, FPC], F32R, isOutput=False)
    out = nc.declare_dram_parameter("out", [HPC, D + 1, S], F32, isOutput=True)

    with tile.TileContext(nc) as tc, ExitStack() as ctx:
        sb = ctx.enter_context(tc.tile_pool(name="sb", bufs=1))
        xs = ctx.enter_context(tc.tile_pool(name="xs", bufs=2))
        exp = ctx.enter_context(tc.tile_pool(name="exp", bufs=2))
        ps = ctx.enter_context(tc.tile_pool(name="ps", bufs=2, space="PSUM"))

        # --- weights, resident ---
        w_sb = {}
        for name, w in (("wq", wq), ("wk", wk), ("wv", wv)):
            t = sb.tile([128, KC, FPC], F32R, name=f"{name}_sb", tag=f"{name}_sb")
            nc.sync.dma_start(out=t, in_=w.rearrange("(kc p) f -> p kc f", p=128))
            w_sb[name] = t

        # --- persistent projection outputs ---
        qT = sb.tile([128, NJ, S], F32R, name="qT", tag="qT")     # [f%128, f//128, s]
        kT = sb.tile([128, NJ, S], F32R, name="kT", tag="kT")
        vaug = sb.tile([128, HPC, NT, D + 1], F32R, name="vaug", tag="vaug")
        for _h in range(HPC):
            for _t in range(NT):
                nc.vector.memset(vaug[:, _h, _t, D:D + 1].bitcast(F32), 1.0)

        # --- q^T / k^T projections ---
        for name, x, dst in (("wq", xq, qT), ("wk", xk, kT)):
            for j in range(NJ):
                xt = xs.tile([128, KC, 512], F32R, name=f"x_{name}_{j}", tag="xt")
                nc.sync.dma_start(
                    out=xt,
                    in_=x[:, j * 512:(j + 1) * 512].rearrange(
                        "(kc p) f -> p kc f", p=128),
                )
                for m in range(4):  # output-feature chunks of 128
                    acc = ps.tile([128, 512], F32, name=f"p_{name}_{j}_{m}",
                                  tag="proj")
                    for kc in range(KC):
                        nc.tensor.matmul(
                            acc,
                            lhsT=w_sb[name][:, kc, m * 128:(m + 1) * 128],
                            rhs=xt[:, kc, :],
                            start=(kc == 0), stop=(kc == KC - 1),
                        )
                    nc.vector.tensor_copy(
                        out=dst[:, m, j * 512:(j + 1) * 512], in_=acc)

        # --- v projection (natural [s, f] layout) into v_aug ---
        for j in range(NJ):
            xt = xs.tile([128, KC, 512], F32R, name=f"x_v_{j}", tag="xt")
            nc.sync.dma_start(
                out=xt,
                in_=xv[:, j * 512:(j + 1) * 512].rearrange(
                    "(kc p) f -> p kc f", p=128),
            )
            for sc in range(4):  # s chunks of 128 inside this j
                t = j * 4 + sc
                acc = ps.tile([128, FPC], F32, name=f"p_v_{j}_{sc}", tag="proj")
                for kc in range(KC):
                    nc.tensor.matmul(
                        acc,
                        lhsT=xt[:, kc, sc * 128:(sc + 1) * 128],
                        rhs=w_sb["wv"][:, kc, :],
                        start=(kc == 0), stop=(kc == KC - 1),
                    )
                for h in range(HPC):
                    nc.vector.tensor_copy(
                        out=vaug[:, h, t, 0:D], in_=acc[:, h * D:(h + 1) * D])

        # --- attention ---
        for h in range(HPC):
            po = (h % 2) * 64   # partition offset of head h inside its chunk
            m = h // 2
            for j in range(NJ):
                cacc = ps.tile([D + 1, 512], F32, name=f"ctx_{h}_{j}", tag="ctx")
                for tg in range(NT // 2):   # exp over 2 banks at a time
                    st = ps.tile([128, 2, 512], F32, name=f"st_{h}_{j}_{tg}",
                                 tag="st")
                    for u in range(2):
                        t = tg * 2 + u
                        nc.tensor.matmul(
                            st[:, u, :],
                            lhsT=kT[po:po + 64, m, t * 128:(t + 1) * 128],
                            rhs=qT[po:po + 64, m, j * 512:(j + 1) * 512],
                            start=True, stop=True,
                        )
                    ex = exp.tile([128, 2, 512], F32R, name=f"ex_{h}_{j}_{tg}",
                                  tag="ex")
                    nc.scalar.activation(ex, st, EXP, scale=SCALE)
                    for u in range(2):
                        t = tg * 2 + u
                        nc.tensor.matmul(
                            cacc,
                            lhsT=vaug[:, h, t, :],
                            rhs=ex[:, u, :],
                            start=(t == 0), stop=(t == NT - 1),
                        )
                csb = exp.tile([D + 1, 512], F32, name=f"csb_{h}_{j}",
                               tag="csb")
                nc.vector.tensor_copy(out=csb, in_=cacc)
                nc.sync.dma_start(
                    out=out[h, :, j * 512:(j + 1) * 512], in_=csb)

    nc.compile()
    nc.freeze()
    return nc


_NC_CACHE = None


def _get_nc():
    global _NC_CACHE
    if _NC_CACHE is None:
        _NC_CACHE = build_bass()
    return _NC_CACHE


def kernel(queries, keys, values, Wq, Wk, Wv, **_):
    queries = np.asarray(queries, dtype=np.float32)
    keys = np.asarray(keys, dtype=np.float32)
    values = np.asarray(values, dtype=np.float32)
    Wq = np.asarray(Wq, dtype=np.float32)
    Wk = np.asarray(Wk, dtype=np.float32)
    Wv = np.asarray(Wv, dtype=np.float32)

    # Host-side shard prep: transpose activations once per batch, slice W by head.
    xq_t = [np.ascontiguousarray(queries[b].T) for b in range(B)]
    xk_t = [np.ascontiguousarray(keys[b].T) for b in range(B)]
    xv_t = [np.ascontiguousarray(values[b].T) for b in range(B)]
    w_half = [
        (np.ascontiguousarray(Wq[:, g * FPC:(g + 1) * FPC]),
         np.ascontiguousarray(Wk[:, g * FPC:(g + 1) * FPC]),
         np.ascontiguousarray(Wv[:, g * FPC:(g + 1) * FPC]))
        for g in range(2)
    ]

    in_maps = []
    for c in range(N_CORES):
        b, g = c // 2, c % 2
        in_maps.append({
            "xq_t": xq_t[b], "xk_t": xk_t[b], "xv_t": xv_t[b],
            "wq": w_half[g][0], "wk": w_half[g][1], "wv": w_half[g][2],
        })

    nc = _get_nc()
    res = run_bass_kernel_spmd(nc, in_maps, list(range(N_CORES)))

    full = np.empty((B, S, H * D), dtype=np.float32)
    for c in range(N_CORES):
        b, g = c // 2, c % 2
        o = res.results[c]["out"]          # [HPC, D+1, S]
        ctx = o[:, :D, :] / o[:, D:D + 1, :]     # [HPC, D, S]
        dst = full[b].reshape(S, H, D)
        dst[:, g * HPC:(g + 1) * HPC, :] = ctx.transpose(2, 0, 1)
    return full
